# revision 43
# baseline (speedup 1.0000x reference)
"""Trainium2 Bass kernel: custom inverse STFT (degenerate per-bin rotation +
Hann window + overlap-add + window correction).

Math (matching the reference):
    F[i,k]  = S_real[i,k]*A[k] + S_imag[i,k]*B[k]
      A[k]  = w[k]*(cos(th)-sin(th))/n,  B[k] = -w[k]*(cos(th)+sin(th))/n
    out[t]  = sum_i F[i, t-256*i] / max(corr[t], 1e-8)

Implementation (fp16 inputs, bf16 products, f32 accumulation):
  - Inputs are cast to fp16 on the host (halves HBM traffic; the 2e-2 rel-err
    budget dwarfs 16-bit rounding).  Coefficients are scaled by 2^8 so
    products stay in fp16's normal range; the host divides the scale out.
  - Sharding: 8192 frames -> 8 cores x 1024 frames (+3 left-halo frames).
  - Per core: 9 slices of 128 consecutive frames starting at multiples of
    125 (slices overlap by 3 frames), frame = partition.  Each slice yields
    125 output blocks whose 4 overlap-add contributions all live on
    partitions of the SAME slice, so the whole overlap-add (including the
    t1+t2 sum) runs on the TensorEngine as shifted-identity matmuls
    accumulating exactly in f32 PSUM.  Products are written as bf16 (PE
    matmul is full-rate for bf16, half-rate for fp16).
  - The host pre-packs the input as [128, 9, 2048] fp16 (slice rows on
    partitions, Sr|Si interleaved, overlap rows duplicated, padding zeroed)
    so every DMA moves 4KB-contiguous per-partition segments, and the DVE
    computes both products of a slice in ONE 2x-packed op against the
    broadcast [A|B] row.
  - Output is stored in device-natural [128, 9*256] order (contiguous per
    partition) and de-interleaved on the host.
  - The 6 edge blocks (corr < 2 there: the window-correction division
    amplifies fp16 underflow by up to ~1e5) and the 768-sample global tail
    are recomputed exactly in f32 on the host.
"""

import numpy as np
import ml_dtypes

import concourse.bass as bass
import concourse.bacc as bacc
import concourse.mybir as mybir
import concourse.tile as tile
from concourse.bass_utils import run_bass_kernel_spmd

F16 = mybir.dt.float16
BF16 = mybir.dt.bfloat16
F32 = mybir.dt.float32
ALU = mybir.AluOpType

P = 128            # SBUF partitions
FL = 1024          # frame length (== fft length)
FS = 256           # frame step
NF = 8192          # total frames
NCORES = 8
FPC = NF // NCORES          # frames owned per core (1024)
ROWS = FPC + 3              # input rows per core (3 left-halo frames)
SL = 125                    # slice stride in frames (overlap of 3)
NS = 8                      # slices x 125 blocks (the last 24 blocks per
                            # core are recomputed on the host, like edges)
W2 = 2 * FL                 # interleaved Sr|Si row width (2048)
OUT_LEN = FS * (NF - 1) + FL
SCALE = np.float32(256.0)   # exact power-of-2 coefficient pre-scale


def _window32():
    # bit-matches the reference's f32 window computation
    k = np.arange(FL, dtype=np.float32)
    th = np.float32(2.0 * np.pi) * k / np.float32(FL)
    return (np.float32(0.5) - np.float32(0.5) * np.cos(th)).astype(np.float32)


def _coeffs32():
    k = np.arange(FL, dtype=np.float64)
    th = 2.0 * np.pi * k / FL
    w = _window32().astype(np.float64)
    a = (w * (np.cos(th) - np.sin(th)) / FL).astype(np.float32)
    b = (-w * (np.cos(th) + np.sin(th)) / FL).astype(np.float32)
    return a, b


def _window_correction():
    w = _window32()
    corr = np.zeros(OUT_LEN, dtype=np.float32)
    for j in range(4):
        view = corr[j * FS:j * FS + NF * FS].reshape(NF, FS)
        view += w[j * FS:(j + 1) * FS][None, :]
    return corr


def build_nc():
    nc = bacc.Bacc(trn_type="TRN2", target_bir_lowering=False, debug=False)
    x_d = nc.dram_tensor("x", [P * NS * W2], F16, kind="ExternalInput").ap()
    cf_d = nc.dram_tensor("coefs", [W2 + P], F16, kind="ExternalInput").ap()
    id_d = nc.dram_tensor("ident", [P, P], BF16, kind="ExternalInput").ap()
    out_d = nc.dram_tensor("out_seg", [P * NS * FS], F16, kind="ExternalOutput").ap()

    xv = x_d.rearrange("(p s k) -> p s k", p=P, s=NS)   # [128, 9, 2048]

    with tile.TileContext(nc) as tc:
        with (
            tc.tile_pool(name="const", bufs=1) as cpool,
            tc.tile_pool(name="main", bufs=1) as mpool,
            tc.tile_pool(name="psum", bufs=1, space="PSUM") as ppool,
            tc.tile_pool(name="psum2", bufs=3, space="PSUM") as qpool,
        ):
            Crow = cpool.tile([1, W2 + P], F16, tag="Crow")  # A*2^8|B*2^8|ones
            Id = cpool.tile([P, P], BF16, tag="Id")
            ABt = cpool.tile([P, W2], F16, tag="ABt")
            X = mpool.tile([P, NS * W2], F16, tag="X")
            T = mpool.tile([P, NS * W2], BF16, tag="T")
            Ot = mpool.tile([P, NS * FS], F16, tag="Ot")
            ABp = ppool.tile([P, W2], F32, tag="ABp")

            # Crow first on sync (feeds the coefficient broadcast), Id on the
            # other queue; the input stream alternates queues so X0 lands ASAP
            nc.sync.dma_start(out=Crow[:, :], in_=cf_d[None, :])
            nc.scalar.dma_start(out=Id[:, :], in_=id_d[:, :])

            # input stream: one 512KB DMA per slice (4KB contiguous per
            # partition), rotating over three DMA queues (2x HWDGE + SWDGE)
            # to close the per-queue completion-to-start gaps
            engs = [nc.sync, nc.scalar, nc.gpsimd]
            for s in range(NS):
                dst = X[:, s * W2:(s + 1) * W2]
                engs[s % 3].dma_start(out=dst, in_=xv[:, s, :])

            # broadcast [A|B] to all partitions via K=1 matmuls (ones @ row)
            # into a 4-bank PSUM tile, then ACT copies per half so the first
            # product can start as soon as the A half is staged
            ones = Crow[0:1, W2:W2 + P]
            for h in range(4):
                nc.tensor.matmul(ABp[:, h * 512:(h + 1) * 512], ones,
                                 Crow[0:1, h * 512:(h + 1) * 512],
                                 start=True, stop=True)
            nc.scalar.copy(out=ABt[:, 0:FL], in_=ABp[:, 0:FL])
            nc.scalar.copy(out=ABt[:, FL:W2], in_=ABp[:, FL:W2])

            # warm-up burst: ungated back-to-back matmuls into a scratch bank
            # trigger the PE clock ramp before the real pair groups arrive
            # (the PE runs at half clock until it sees sustained work)
            Wp = ppool.tile([P, 512], F32, tag="Wp")
            for _ in range(12):
                nc.tensor.matmul(Wp[:, :], ones, Crow[0:1, 0:512],
                                 start=True, stop=True)

            # products on the DVE (fp16 x fp16 -> bf16, 2x packed mode):
            # slice 0 split per half (starts right after the A staging);
            # remaining slices in one op over [Sr|Si] x [A|B]
            nc.vector.tensor_tensor(out=T[:, 0:FL], in0=X[:, 0:FL],
                                    in1=ABt[:, 0:FL], op=ALU.mult)
            nc.vector.tensor_tensor(out=T[:, FL:W2], in0=X[:, FL:W2],
                                    in1=ABt[:, FL:W2], op=ALU.mult)
            for s in range(1, NS):
                sl = slice(s * W2, (s + 1) * W2)
                nc.vector.tensor_tensor(out=T[:, sl], in0=X[:, sl],
                                        in1=ABt[:, :], op=ALU.mult)

            # overlap-add on the PE: slice s, output block m (0..124), chunk c
            # reads frame q = m+3-c of the same slice -> lhsT = Id[:, 3-c:128-c]
            Tv = T[:, :].rearrange("p (s x) -> p s x", s=NS)
            ov = out_d.rearrange("(p x) -> p x", p=P)
            pairs = [(0, 2), (2, 2), (4, 2), (6, 2)]
            # store each region (all 128 partitions -> full SDMA spread) as
            # soon as its pairs are staged; the host de-interleaves blocks
            out_after = {1: slice(0, 4 * FS), 2: slice(4 * FS, 6 * FS),
                         3: slice(6 * FS, 8 * FS)}
            for i, (s0, nsl) in enumerate(pairs):
                osl = slice(s0 * FS, (s0 + nsl) * FS)
                Opt = qpool.tile([P, 512], F32, tag="Opt")
                if i < 3:
                    # one 8-matmul group per pair (N=512 streams best)
                    for c in range(4):
                        w = Id[:, 3 - c:3 - c + SL]
                        k1 = slice(c * FS, (c + 1) * FS)
                        k2 = slice(FL + c * FS, FL + (c + 1) * FS)
                        nc.tensor.matmul(Opt[0:SL, 0:nsl * FS], w,
                                         Tv[:, s0:s0 + nsl, k1],
                                         start=(c == 0), stop=False)
                        nc.tensor.matmul(Opt[0:SL, 0:nsl * FS], w,
                                         Tv[:, s0:s0 + nsl, k2],
                                         start=False, stop=(c == 3))
                else:
                    # last pair split per slice so only slice-7's 8 small
                    # matmuls remain after the final input slice lands
                    for j in range(nsl):
                        psl = slice(j * FS, (j + 1) * FS)
                        for c in range(4):
                            w = Id[:, 3 - c:3 - c + SL]
                            k1 = slice(c * FS, (c + 1) * FS)
                            k2 = slice(FL + c * FS, FL + (c + 1) * FS)
                            nc.tensor.matmul(Opt[0:SL, psl], w,
                                             Tv[:, s0 + j, k1],
                                             start=(c == 0), stop=False)
                            nc.tensor.matmul(Opt[0:SL, psl], w,
                                             Tv[:, s0 + j, k2],
                                             start=False, stop=(c == 3))
                # PSUM -> SBUF fp16 staging on the ACT engine
                nc.scalar.copy(out=Ot[0:SL, osl], in_=Opt[0:SL, 0:nsl * FS])
                if i in out_after:
                    o = out_after[i]
                    nc.scalar.dma_start(out=ov[:, o], in_=Ot[:, o])

            # (store DMAs are interleaved into the pair loop above via the
            # out_chunks schedule below)
    nc.compile()
    return nc


_cache = {}


def _get_nc():
    if "nc" not in _cache:
        _cache["nc"] = build_nc()
    return _cache["nc"]


def make_in_maps(S_real, S_imag):
    a32, b32 = _coeffs32()
    coefs = np.zeros(W2 + P, dtype=np.float16)
    coefs[0:FL] = (a32 * SCALE).astype(np.float16)
    coefs[FL:W2] = (b32 * SCALE).astype(np.float16)
    coefs[W2:] = np.float16(1.0)
    ident = np.eye(P, dtype=ml_dtypes.bfloat16)

    # interleaved + padded fp16 input: row r of core m = global frame
    # m*1024 - 3 + r (zeros outside [0, NF))
    sr16 = S_real.astype(np.float16)
    si16 = S_imag.astype(np.float16)
    glob = np.zeros((3 + NF + P, W2), dtype=np.float16)
    glob[3:3 + NF, 0:FL] = sr16
    glob[3:3 + NF, FL:W2] = si16

    in_maps = []
    for m in range(NCORES):
        base = m * FPC
        x = np.empty((P, NS, W2), dtype=np.float16)
        for s in range(NS):
            x[:, s, :] = glob[base + s * SL:base + s * SL + P]
        in_maps.append({
            "x": x.reshape(-1),
            "coefs": coefs,
            "ident": ident,
        })
    return in_maps


def assemble_output(S_real, S_imag, segs):
    a32, b32 = _coeffs32()
    full = np.empty(OUT_LEN, dtype=np.float32)
    inv_scale = np.float32(1.0) / SCALE
    for m in range(NCORES):
        # seg[p, s*256+r] -> block s*125+p (1000 blocks from the device)
        v = segs[m].reshape(P, NS, FS)[0:SL].transpose(1, 0, 2).reshape(-1)
        bt = m * FPC * FS
        full[bt:bt + NS * SL * FS] = v.astype(np.float32) * inv_scale
        # the core's last 24 blocks: exact f32 on the host
        f0 = m * FPC + 997
        Fr = S_real[f0:f0 + 27] * a32[None, :] + S_imag[f0:f0 + 27] * b32[None, :]
        for lb in range(NS * SL, FPC):
            acc = Fr[lb - 997, 0:FS].copy()
            for c in range(1, 4):
                acc += Fr[lb - c - 997, c * FS:(c + 1) * FS]
            full[(m * FPC + lb) * FS:(m * FPC + lb + 1) * FS] = acc

    # exact f32 recompute of the 6 edge blocks (corr < 2 there: the final
    # division amplifies fp16 error by up to ~1e5) and the global tail
    Fh = S_real[0:3] * a32[None, :] + S_imag[0:3] * b32[None, :]
    full[0:FS] = Fh[0, 0:FS]
    full[FS:2 * FS] = Fh[0, FS:2 * FS] + Fh[1, 0:FS]
    full[2 * FS:3 * FS] = Fh[0, 2 * FS:3 * FS] + Fh[1, FS:2 * FS] + Fh[2, 0:FS]
    Ft = S_real[NF - 3:] * a32[None, :] + S_imag[NF - 3:] * b32[None, :]
    full[NF * FS:NF * FS + FS] = Ft[0, 3 * FS:] + Ft[1, 2 * FS:3 * FS] + Ft[2, FS:2 * FS]
    full[NF * FS + FS:NF * FS + 2 * FS] = Ft[1, 3 * FS:] + Ft[2, 2 * FS:3 * FS]
    full[NF * FS + 2 * FS:] = Ft[2, 3 * FS:]

    if "corr" not in _cache:
        _cache["corr"] = np.maximum(_window_correction(), np.float32(1e-8))
    return full / _cache["corr"]


def kernel(S_real, S_imag):
    S_real = np.asarray(S_real, dtype=np.float32)
    S_imag = np.asarray(S_imag, dtype=np.float32)
    in_maps = make_in_maps(S_real, S_imag)
    nc = _get_nc()
    res = run_bass_kernel_spmd(nc, in_maps, list(range(NCORES)))
    segs = [res.results[m]["out_seg"] for m in range(NCORES)]
    return assemble_output(S_real, S_imag, segs)


# revision 46
# speedup vs baseline: 1.0559x; 1.0559x over previous
"""Trainium2 Bass kernel: custom inverse STFT (degenerate per-bin rotation +
Hann window + overlap-add + window correction).

Math (matching the reference):
    F[i,k]  = S_real[i,k]*A[k] + S_imag[i,k]*B[k]
      A[k]  = w[k]*(cos(th)-sin(th))/n,  B[k] = -w[k]*(cos(th)+sin(th))/n
    out[t]  = sum_i F[i, t-256*i] / max(corr[t], 1e-8)

Implementation (fp16 inputs, bf16 products, f32 accumulation):
  - Inputs are cast to fp16 on the host (halves HBM traffic; the 2e-2 rel-err
    budget dwarfs 16-bit rounding).  Coefficients are scaled by 2^8 so
    products stay in fp16's normal range; the host divides the scale out.
  - Sharding: 8192 frames -> 8 cores x 1024 frames (+3 left-halo frames).
  - Per core: 9 slices of 128 consecutive frames starting at multiples of
    125 (slices overlap by 3 frames), frame = partition.  Each slice yields
    125 output blocks whose 4 overlap-add contributions all live on
    partitions of the SAME slice, so the whole overlap-add (including the
    t1+t2 sum) runs on the TensorEngine as shifted-identity matmuls
    accumulating exactly in f32 PSUM.  Products are written as bf16 (PE
    matmul is full-rate for bf16, half-rate for fp16).
  - The host pre-packs the input as [128, 9, 2048] fp16 (slice rows on
    partitions, Sr|Si interleaved, overlap rows duplicated, padding zeroed)
    so every DMA moves 4KB-contiguous per-partition segments, and the DVE
    computes both products of a slice in ONE 2x-packed op against the
    broadcast [A|B] row.
  - Output is stored in device-natural [128, 9*256] order (contiguous per
    partition) and de-interleaved on the host.
  - The 6 edge blocks (corr < 2 there: the window-correction division
    amplifies fp16 underflow by up to ~1e5) and the 768-sample global tail
    are recomputed exactly in f32 on the host.
"""

import numpy as np
import ml_dtypes

import concourse.bass as bass
import concourse.bacc as bacc
import concourse.mybir as mybir
import concourse.tile as tile
from concourse.bass_utils import run_bass_kernel_spmd

F16 = mybir.dt.float16
BF16 = mybir.dt.bfloat16
F32 = mybir.dt.float32
ALU = mybir.AluOpType

P = 128            # SBUF partitions
FL = 1024          # frame length (== fft length)
FS = 256           # frame step
NF = 8192          # total frames
NCORES = 8
FPC = NF // NCORES          # frames owned per core (1024)
ROWS = FPC + 3              # input rows per core (3 left-halo frames)
SL = 125                    # slice stride in frames (overlap of 3)
NS = 8                      # slices x 125 blocks (the last 24 blocks per
                            # core are recomputed on the host, like edges)
W2 = 2 * FL                 # interleaved Sr|Si row width (2048)
OUT_LEN = FS * (NF - 1) + FL
SCALE = np.float32(256.0)   # exact power-of-2 coefficient pre-scale


def _window32():
    # bit-matches the reference's f32 window computation
    k = np.arange(FL, dtype=np.float32)
    th = np.float32(2.0 * np.pi) * k / np.float32(FL)
    return (np.float32(0.5) - np.float32(0.5) * np.cos(th)).astype(np.float32)


def _coeffs32():
    k = np.arange(FL, dtype=np.float64)
    th = 2.0 * np.pi * k / FL
    w = _window32().astype(np.float64)
    a = (w * (np.cos(th) - np.sin(th)) / FL).astype(np.float32)
    b = (-w * (np.cos(th) + np.sin(th)) / FL).astype(np.float32)
    return a, b


def _window_correction():
    w = _window32()
    corr = np.zeros(OUT_LEN, dtype=np.float32)
    for j in range(4):
        view = corr[j * FS:j * FS + NF * FS].reshape(NF, FS)
        view += w[j * FS:(j + 1) * FS][None, :]
    return corr


def build_nc():
    nc = bacc.Bacc(trn_type="TRN2", target_bir_lowering=False, debug=False)
    x_d = nc.dram_tensor("x", [P * NS * W2], F16, kind="ExternalInput").ap()
    cf_d = nc.dram_tensor("coefs", [W2 + P], F16, kind="ExternalInput").ap()
    id_d = nc.dram_tensor("ident", [P, P], BF16, kind="ExternalInput").ap()
    out_d = nc.dram_tensor("out_seg", [P * NS * FS], F16, kind="ExternalOutput").ap()

    xv = x_d.rearrange("(p s k) -> p s k", p=P, s=NS)   # [128, 9, 2048]

    with tile.TileContext(nc) as tc:
        with (
            tc.tile_pool(name="const", bufs=1) as cpool,
            tc.tile_pool(name="main", bufs=1) as mpool,
            tc.tile_pool(name="psum", bufs=1, space="PSUM") as ppool,
            tc.tile_pool(name="psum2", bufs=3, space="PSUM") as qpool,
        ):
            Crow = cpool.tile([1, W2 + P], F16, tag="Crow")  # A*2^8|B*2^8|ones
            Id = cpool.tile([P, P], BF16, tag="Id")
            ABt = cpool.tile([P, W2], F16, tag="ABt")
            X = mpool.tile([P, NS * W2], F16, tag="X")
            T = mpool.tile([P, NS * W2], BF16, tag="T")
            Ot = mpool.tile([P, NS * FS], F16, tag="Ot")
            ABp = ppool.tile([P, W2], F32, tag="ABp")

            # Crow first on sync (feeds the coefficient broadcast); Id rides
            # the sync queue after X0 so the scalar queue starts X1 at once
            # and X7 (the tail gate) lands a touch earlier
            nc.sync.dma_start(out=Crow[:, :], in_=cf_d[None, :])

            # input stream: one 512KB DMA per slice (4KB contiguous per
            # partition), alternating between the two HWDGE queues
            for s in range(NS):
                dst = X[:, s * W2:(s + 1) * W2]
                eng = nc.sync if s % 2 == 0 else nc.scalar
                eng.dma_start(out=dst, in_=xv[:, s, :])
                if s == 0:
                    nc.sync.dma_start(out=Id[:, :], in_=id_d[:, :])

            # broadcast [A|B] to all partitions via K=1 matmuls (ones @ row)
            # into a 4-bank PSUM tile, then ACT copies per half so the first
            # product can start as soon as the A half is staged
            ones = Crow[0:1, W2:W2 + P]
            for h in range(4):
                nc.tensor.matmul(ABp[:, h * 512:(h + 1) * 512], ones,
                                 Crow[0:1, h * 512:(h + 1) * 512],
                                 start=True, stop=True)
            nc.scalar.copy(out=ABt[:, 0:FL], in_=ABp[:, 0:FL])
            nc.scalar.copy(out=ABt[:, FL:W2], in_=ABp[:, FL:W2])

            # warm-up burst: ungated back-to-back matmuls into a scratch bank
            # trigger the PE clock ramp before the real pair groups arrive
            # (the PE runs at half clock until it sees sustained work)
            Wp = ppool.tile([P, 512], F32, tag="Wp")
            for _ in range(8):
                nc.tensor.matmul(Wp[:, :], ones, Crow[0:1, 0:512],
                                 start=True, stop=True)

            # products on the DVE (fp16 x fp16 -> bf16, 2x packed mode):
            # slice 0 split per half (starts right after the A staging);
            # remaining slices in one op over [Sr|Si] x [A|B]
            nc.vector.tensor_tensor(out=T[:, 0:FL], in0=X[:, 0:FL],
                                    in1=ABt[:, 0:FL], op=ALU.mult)
            nc.vector.tensor_tensor(out=T[:, FL:W2], in0=X[:, FL:W2],
                                    in1=ABt[:, FL:W2], op=ALU.mult)
            for s in range(1, NS):
                sl = slice(s * W2, (s + 1) * W2)
                nc.vector.tensor_tensor(out=T[:, sl], in0=X[:, sl],
                                        in1=ABt[:, :], op=ALU.mult)

            # overlap-add on the PE: slice s, output block m (0..124), chunk c
            # reads frame q = m+3-c of the same slice -> lhsT = Id[:, 3-c:128-c]
            Tv = T[:, :].rearrange("p (s x) -> p s x", s=NS)
            ov = out_d.rearrange("(p x) -> p x", p=P)
            pairs = [(0, 2), (2, 2), (4, 2), (6, 2)]
            # store each region (all 128 partitions -> full SDMA spread) as
            # soon as its pairs are staged; the host de-interleaves blocks
            out_after = {1: slice(0, 4 * FS), 3: slice(4 * FS, 8 * FS)}
            for i, (s0, nsl) in enumerate(pairs):
                osl = slice(s0 * FS, (s0 + nsl) * FS)
                Opt = qpool.tile([P, 512], F32, tag="Opt")
                if i < 3:
                    # one 8-matmul group per pair (N=512 streams best)
                    for c in range(4):
                        w = Id[:, 3 - c:3 - c + SL]
                        k1 = slice(c * FS, (c + 1) * FS)
                        k2 = slice(FL + c * FS, FL + (c + 1) * FS)
                        nc.tensor.matmul(Opt[0:SL, 0:nsl * FS], w,
                                         Tv[:, s0:s0 + nsl, k1],
                                         start=(c == 0), stop=False)
                        nc.tensor.matmul(Opt[0:SL, 0:nsl * FS], w,
                                         Tv[:, s0:s0 + nsl, k2],
                                         start=False, stop=(c == 3))
                else:
                    # last pair split per slice so only slice-7's 8 small
                    # matmuls remain after the final input slice lands
                    for j in range(nsl):
                        psl = slice(j * FS, (j + 1) * FS)
                        for c in range(4):
                            w = Id[:, 3 - c:3 - c + SL]
                            k1 = slice(c * FS, (c + 1) * FS)
                            k2 = slice(FL + c * FS, FL + (c + 1) * FS)
                            nc.tensor.matmul(Opt[0:SL, psl], w,
                                             Tv[:, s0 + j, k1],
                                             start=(c == 0), stop=False)
                            nc.tensor.matmul(Opt[0:SL, psl], w,
                                             Tv[:, s0 + j, k2],
                                             start=False, stop=(c == 3))
                # PSUM -> SBUF fp16 staging on the ACT engine
                nc.scalar.copy(out=Ot[0:SL, osl], in_=Opt[0:SL, 0:nsl * FS])
                if i in out_after:
                    o = out_after[i]
                    nc.scalar.dma_start(out=ov[:, o], in_=Ot[:, o])

            # (store DMAs are interleaved into the pair loop above via the
            # out_chunks schedule below)
    nc.compile()
    return nc


_cache = {}


def _get_nc():
    if "nc" not in _cache:
        _cache["nc"] = build_nc()
    return _cache["nc"]


def make_in_maps(S_real, S_imag):
    a32, b32 = _coeffs32()
    coefs = np.zeros(W2 + P, dtype=np.float16)
    coefs[0:FL] = (a32 * SCALE).astype(np.float16)
    coefs[FL:W2] = (b32 * SCALE).astype(np.float16)
    coefs[W2:] = np.float16(1.0)
    ident = np.eye(P, dtype=ml_dtypes.bfloat16)

    # interleaved + padded fp16 input: row r of core m = global frame
    # m*1024 - 3 + r (zeros outside [0, NF))
    sr16 = S_real.astype(np.float16)
    si16 = S_imag.astype(np.float16)
    glob = np.zeros((3 + NF + P, W2), dtype=np.float16)
    glob[3:3 + NF, 0:FL] = sr16
    glob[3:3 + NF, FL:W2] = si16

    in_maps = []
    for m in range(NCORES):
        base = m * FPC
        x = np.empty((P, NS, W2), dtype=np.float16)
        for s in range(NS):
            x[:, s, :] = glob[base + s * SL:base + s * SL + P]
        in_maps.append({
            "x": x.reshape(-1),
            "coefs": coefs,
            "ident": ident,
        })
    return in_maps


def assemble_output(S_real, S_imag, segs):
    a32, b32 = _coeffs32()
    full = np.empty(OUT_LEN, dtype=np.float32)
    inv_scale = np.float32(1.0) / SCALE
    for m in range(NCORES):
        # seg[p, s*256+r] -> block s*125+p (1000 blocks from the device)
        v = segs[m].reshape(P, NS, FS)[0:SL].transpose(1, 0, 2).reshape(-1)
        bt = m * FPC * FS
        full[bt:bt + NS * SL * FS] = v.astype(np.float32) * inv_scale
        # the core's last 24 blocks: exact f32 on the host
        f0 = m * FPC + 997
        Fr = S_real[f0:f0 + 27] * a32[None, :] + S_imag[f0:f0 + 27] * b32[None, :]
        for lb in range(NS * SL, FPC):
            acc = Fr[lb - 997, 0:FS].copy()
            for c in range(1, 4):
                acc += Fr[lb - c - 997, c * FS:(c + 1) * FS]
            full[(m * FPC + lb) * FS:(m * FPC + lb + 1) * FS] = acc

    # exact f32 recompute of the 6 edge blocks (corr < 2 there: the final
    # division amplifies fp16 error by up to ~1e5) and the global tail
    Fh = S_real[0:3] * a32[None, :] + S_imag[0:3] * b32[None, :]
    full[0:FS] = Fh[0, 0:FS]
    full[FS:2 * FS] = Fh[0, FS:2 * FS] + Fh[1, 0:FS]
    full[2 * FS:3 * FS] = Fh[0, 2 * FS:3 * FS] + Fh[1, FS:2 * FS] + Fh[2, 0:FS]
    Ft = S_real[NF - 3:] * a32[None, :] + S_imag[NF - 3:] * b32[None, :]
    full[NF * FS:NF * FS + FS] = Ft[0, 3 * FS:] + Ft[1, 2 * FS:3 * FS] + Ft[2, FS:2 * FS]
    full[NF * FS + FS:NF * FS + 2 * FS] = Ft[1, 3 * FS:] + Ft[2, 2 * FS:3 * FS]
    full[NF * FS + 2 * FS:] = Ft[2, 3 * FS:]

    if "corr" not in _cache:
        _cache["corr"] = np.maximum(_window_correction(), np.float32(1e-8))
    return full / _cache["corr"]


def kernel(S_real, S_imag):
    S_real = np.asarray(S_real, dtype=np.float32)
    S_imag = np.asarray(S_imag, dtype=np.float32)
    in_maps = make_in_maps(S_real, S_imag)
    nc = _get_nc()
    res = run_bass_kernel_spmd(nc, in_maps, list(range(NCORES)))
    segs = [res.results[m]["out_seg"] for m in range(NCORES)]
    return assemble_output(S_real, S_imag, segs)
